# revision 32
# baseline (speedup 1.0000x reference)
"""BinLinear (sign-quantized linear) Trainium2 kernel, mixed bf16/fp8.

Computes out = x @ sign(clip(w, -1, 1)).T for x[8192, 4096], w[4096, 4096],
data-parallel over the 8 NeuronCores (each core takes 1024 rows of x and the
full weight matrix).

The PE moving-operand rate is one column per cycle regardless of dtype, so
the bf16 baseline (~462us) sits at the bf16 roofline. The only faster mode on
TRN2 is fp8 DoubleRow (2 fp8 weights per cell, 256-deep contraction per
matmul, measured at the same 216ns issue gap as a 128-deep bf16 matmul =
2x MACs/cycle). Pure e4m3 x quantization gives 2.64e-2 relative error --
over the 2e-2 budget -- so the contraction is split:

  - KB=16 of the 32 k-tiles (128 each) run in bf16: x bf16 stationary,
    sign(w) fp8 moving (fp8 moving runs at bf16 speed and halves weight DMA;
    sign() is computed on host so no on-device activation is needed).
  - the remaining 16 k-tiles run as 8 fp8 DoubleRow pair-tiles: x e4m3
    stationary [128,2,128], sign(w) e4m3 moving [128,2,512] (+-1 exact).

Exact full-output error (deterministic): rel_norm 1.881e-2, absmax-rel
1.46e-2, vs the 2e-2 gate.

Schedule: 7 warmup matmuls on a zeroed tile (memset on the otherwise-idle
Vector engine) bridge the initial DMA wait so the HAM clock-gate (half rate
until ~3.4us of sustained PE activity) is at 8/8 when the real stream
begins; bf16 x tiles load as two 512-column halves (1KB DMA packets run
~40% faster than the 512B packets smaller chunks produce) alternating
between the Activation and sync DGE queues so neither queue's ~150GB/s
saturates during ob0; weight tiles stream on the sync DGE queue with 4-tile
lookahead; odd output blocks run their pair-tiles in reverse (DR first) so
consecutive blocks share the PE perf mode at the boundary -- each bf16<->DR
switch costs ~200-600ns of PE pipeline drain; PSUM drains alternate
VectorE/ScalarE and their output DMAs use the gpsimd SWDGE queue so they
never block weight fetches; outputs ship as bf16 (host upcasts, ~1e-3 error
contribution); the final output block runs from prefetched SBUF tiles as a
DR pass opening all 8 bank chains (seamless from ob6) then a bf16 pass
closing them with drains spread across the pass, and the last bank is two
complete 256-column chains (two interleaved open accumulation groups in
one bank corrupt results, so each half runs start->stop by itself) whose
final drain splits across both copy engines and both HW DGE queues so only
~2x32KB of parallel copy+DMA trails the final matmul.
"""

import numpy as np
import ml_dtypes

import concourse.mybir as mybir
import concourse.tile as tile
from concourse import bacc
from concourse.bass_utils import run_bass_kernel_spmd

N_CORES = 8
N_FULL, IN_CH, OUT_CH = 8192, 4096, 4096
P = 128
OBLK = 512
KB = 16  # k-tiles (of 32) done in bf16; must be even. rest = fp8 DoubleRow.

DR = mybir.MatmulPerfMode.DoubleRow
COPY_FN = mybir.ActivationFunctionType.Copy


def build_nc(ns, in_ch, out_ch, kb):
    kt = in_ch // P        # total 128-k-tiles (32)
    tp = kt // 2           # pair tiles (16)
    tb = kb // 2           # pair tiles fully in bf16
    ndr = tp - tb          # DoubleRow pair tiles
    msub = ns // P         # PSUM banks in flight (8)
    nob = out_ch // OBLK   # output-channel blocks (8)
    assert kb % 2 == 0 and msub <= 8

    nc = bacc.Bacc("TRN2", target_bir_lowering=False, debug=False)
    if kb > 0:
        xb_d = nc.dram_tensor("xb", [kb * P, ns], mybir.dt.bfloat16,
                              kind="ExternalInput")
    if ndr > 0:
        xq_d = nc.dram_tensor("xq", [ndr * P, 2, ns], mybir.dt.float8e4,
                              kind="ExternalInput")
    w_d = nc.dram_tensor("w", [tp * P, 2, out_ch], mybir.dt.float8e4,
                         kind="ExternalInput")
    out_d = nc.dram_tensor("out", [ns, out_ch], mybir.dt.bfloat16,
                           kind="ExternalOutput")

    with tile.TileContext(nc) as tc:
        with (
            tc.tile_pool(name="xbpool", bufs=1) as xbpool,
            tc.tile_pool(name="xqpool", bufs=1) as xqpool,
            tc.tile_pool(name="wpool", bufs=20) as wpool,
            tc.tile_pool(name="wlpool", bufs=1) as wlpool,
            tc.tile_pool(name="opool", bufs=8) as opool,
            tc.tile_pool(name="const", bufs=1) as const,
            tc.tile_pool(name="pspool", bufs=1, space="PSUM") as pspool,
        ):
            # HAM warmup: self-contained matmuls on a zeroed tile run during
            # the initial input-DMA wait so the PE clock-gate (4/8 = half
            # rate until ~3.4us of sustained activity) reaches 8/8 before
            # the real stream begins. The memset rides the Vector engine
            # (idle until the first drain) so the warmups start as early as
            # possible after the framework preamble; 7 warmups keep the PE
            # busy through the DMA spin-up window so the gate never
            # re-throttles.
            wz = const.tile([P, OBLK], mybir.dt.bfloat16, name="wz")
            nc.any.memset(wz[:], 0.0)
            warm_ps = pspool.tile([P, OBLK], mybir.dt.float32, name="ps_7")
            for _ in range(7):
                nc.tensor.matmul(warm_ps[:], wz[:, :P], wz[:],
                                 start=True, stop=True)

            x_b = [None] * kb    # bf16 x tiles [P, ns]
            x_q = [None] * ndr   # fp8 pair x tiles [P, 2, ns]
            wlast = [None] * tp  # prefetched last-block w tiles

            def load_x(t):
                # lazily load the x tiles needed by pair-tile t. Loads go on
                # the Activation engine's HW DGE queue so ob0's x streaming
                # never delays weight fetches on the sync queue. bf16 tiles
                # load as two 512-column halves: DMA engine rate is packet-
                # size-bound (1KB packets ~210GB/s aggregate vs 512B ~150),
                # and half-tile granularity keeps arrival ahead of the
                # matmul stream from the very first tile.
                if t >= tp:
                    return
                if t < tb:
                    for j in (0, 1):
                        k = 2 * t + j
                        if x_b[k] is None:
                            xt = xbpool.tile([P, ns], mybir.dt.bfloat16,
                                             name=f"xb_{k}")
                            eng = nc.sync if k % 2 == 1 else nc.scalar
                            cw = ns // 2
                            for c in range(2):
                                eng.dma_start(
                                    out=xt[:, c * cw:(c + 1) * cw],
                                    in_=xb_d[k * P:(k + 1) * P,
                                             c * cw:(c + 1) * cw])
                            x_b[k] = xt
                else:
                    td = t - tb
                    if x_q[td] is None:
                        xt = xqpool.tile([P, 2, ns], mybir.dt.float8e4,
                                         name=f"xq_{td}")
                        nc.scalar.dma_start(
                            out=xt[:], in_=xq_d[td * P:(td + 1) * P, :, :])
                        x_q[td] = xt

            def drain(m, psum, c0):
                # copies alternate VectorE/ScalarE so the 8-bank chain halves
                # (splitting each bank across BOTH engines measures worse --
                # it doubles each engine's copy-queue depth and delays the
                # late banks); output DMAs ride the gpsimd SWDGE queue so a
                # drain waiting on its copy never blocks weight fetches on
                # the sync queue
                ot = opool.tile([P, OBLK], mybir.dt.bfloat16, name="ot")
                if m % 2 == 1:
                    nc.scalar.activation(ot[:], psum[:], COPY_FN)
                else:
                    nc.vector.tensor_copy(ot[:], psum[:])
                nc.gpsimd.dma_start(
                    out=out_d[m * P:(m + 1) * P, c0:c0 + OBLK],
                    in_=ot[:])

            # flat (ob, t) job list with w DMAs emitted 4 tiles ahead so the
            # next block's first weight fetches are queued before the current
            # block's drains. Odd blocks run their pair-tiles in reverse
            # (DR first) so consecutive blocks share the PE perf mode at the
            # boundary: each bf16<->DR switch costs ~200-600ns of PE pipeline
            # drain, and alternating halves the switch count.
            jobs = [(ob, t)
                    for ob in range(nob - 1)
                    for t in (range(tp) if ob % 2 == 0
                              else range(tp - 1, -1, -1))]
            wtiles = {}

            def fetch(idx):
                if idx < len(jobs):
                    ob, t = jobs[idx]
                    wt = wpool.tile([P, 2, OBLK], mybir.dt.float8e4, name="wt")
                    nc.sync.dma_start(
                        out=wt[:],
                        in_=w_d[t * P:(t + 1) * P, :, ob * OBLK:(ob + 1) * OBLK])
                    wtiles[(ob, t)] = wt

            fetch(0)
            load_x(0)
            load_x(1)
            for i in range(1, 4):
                fetch(i)

            psums = None
            for idx, (ob, t) in enumerate(jobs):
                c0 = ob * OBLK
                first = idx % tp == 0       # first pair-tile of the block
                last = idx % tp == tp - 1   # last pair-tile of the block
                if first:
                    psums = [
                        pspool.tile([P, OBLK], mybir.dt.float32, name=f"ps_{m}")
                        for m in range(msub)
                    ]
                fetch(idx + 4)
                if ob == 0:
                    load_x(t + 2)
                wt = wtiles.pop((ob, t))
                if ob == nob - 3:
                    # prefetch last block's w two blocks early (ACT DGE queue) so
                    # its consolidated DMA wait resolves well before the last
                    # block starts m-outer from SBUF
                    wl = wlpool.tile([P, 2, OBLK], mybir.dt.float8e4,
                                     name=f"wl_{t}")
                    nc.scalar.dma_start(
                        out=wl[:],
                        in_=w_d[t * P:(t + 1) * P, :, (nob - 1) * OBLK:])
                    wlast[t] = wl
                if t < tb:
                    for j in (0, 1):
                        k = 2 * t + j
                        for m in range(msub):
                            nc.tensor.matmul(
                                psums[m][:],
                                x_b[k][:, m * P:(m + 1) * P],
                                wt[:, j:j + 1, :],
                                start=(first and j == 0),
                                stop=(last and j == 1),
                            )
                else:
                    td = t - tb
                    for m in range(msub):
                        nc.tensor.matmul(
                            psums[m][:],
                            x_q[td][:, :, m * P:(m + 1) * P],
                            wt[:],
                            start=first,
                            stop=last,
                            perf_mode=DR,
                        )
                if last:
                    for m in range(msub):
                        drain(m, psums[m], c0)

            # Last block, from prefetched SBUF tiles, as TWO passes over the
            # PSUM regions: a DR pass (seamless entry -- block ob=6 ends in
            # DR mode) opening all chains, then a bf16 pass closing them,
            # with each region's drain right after its bf16 chain so drains
            # spread across the pass. One perf-mode switch for the whole
            # block instead of one per m-group. The last bank is split into
            # two 256-column regions so only a 64KB copy+DMA trails the
            # final matmul (256 is the narrowest width whose matmuls stay
            # ahead of their LDWEIGHTS); its output DMAs ride the
            # low-latency sync queue (weights are done by now).
            c0 = (nob - 1) * OBLK
            psums = [
                pspool.tile([P, OBLK], mybir.dt.float32, name=f"ps_{m}")
                for m in range(msub)
            ]
            h = OBLK // 2

            def last_dr(ps_ap, m, lo, hi, start, stop):
                for td in range(ndr):
                    nc.tensor.matmul(
                        ps_ap,
                        x_q[td][:, :, m * P:(m + 1) * P],
                        wlast[tb + td][:, :, lo:hi],
                        start=(start and td == 0),
                        stop=(stop and td == ndr - 1),
                        perf_mode=DR,
                    )

            def last_bf(ps_ap, m, lo, hi, start, stop):
                for t in range(tb):
                    for j in (0, 1):
                        k = 2 * t + j
                        nc.tensor.matmul(
                            ps_ap,
                            x_b[k][:, m * P:(m + 1) * P],
                            wlast[t][:, j:j + 1, lo:hi],
                            start=(start and t == 0 and j == 0),
                            stop=(stop and t == tb - 1 and j == 1),
                        )

            # full banks m0..6: DR pass opens all chains (seamless entry --
            # block ob=6 ends in DR mode), bf16 pass closes them with each
            # bank's drain right after its chain so drains spread out
            for m in range(msub - 1):
                last_dr(psums[m][:, :], m, 0, OBLK, start=True, stop=False)
            for m in range(msub - 1):
                last_bf(psums[m][:, :], m, 0, OBLK, start=False, stop=True)
                drain(m, psums[m], c0)
            # bank 7: two complete 256-column chains (interleaving two open
            # accumulation groups in ONE bank corrupts the first's results,
            # so each half runs start->stop by itself). Chain A runs
            # bf16-first (seamless from the bf16 pass), chain B DR-first
            # (seamless from A), so only a 64KB copy+DMA on the sync queue
            # trails the final matmul and B's tail runs at bf16 cadence.
            mlast = msub - 1
            last_bf(psums[mlast][:, :h], mlast, 0, h, start=True, stop=False)
            last_dr(psums[mlast][:, :h], mlast, 0, h, start=False, stop=True)
            ot_a = opool.tile([P, h], mybir.dt.bfloat16, name="ot_a")
            nc.vector.tensor_copy(ot_a[:], psums[mlast][:, :h])
            nc.sync.dma_start(
                out=out_d[mlast * P:(mlast + 1) * P, c0:c0 + h],
                in_=ot_a[:])
            last_dr(psums[mlast][:, h:], mlast, h, OBLK, start=True, stop=False)
            last_bf(psums[mlast][:, h:], mlast, h, OBLK, start=False, stop=True)
            # the very last drain splits across both copy engines with two
            # pipelined sync DMAs so the post-final-matmul tail is minimal
            q = h // 2
            ot_b = opool.tile([P, h], mybir.dt.bfloat16, name="ot_b")
            nc.vector.tensor_copy(ot_b[:, :q], psums[mlast][:, h:h + q])
            nc.scalar.activation(ot_b[:, q:], psums[mlast][:, h + q:], COPY_FN)
            nc.sync.dma_start(
                out=out_d[mlast * P:(mlast + 1) * P, c0 + h:c0 + h + q],
                in_=ot_b[:, :q])
            nc.scalar.dma_start(
                out=out_d[mlast * P:(mlast + 1) * P, c0 + h + q:c0 + OBLK],
                in_=ot_b[:, q:])
    nc.compile()
    return nc


def prep_in_maps(x, weights_real, kb, n_cores=N_CORES):
    x = np.asarray(x, dtype=np.float32)
    w = np.asarray(weights_real, dtype=np.float32)
    ns = x.shape[0] // n_cores
    in_ch = x.shape[1]
    kt = in_ch // P
    tp = kt // 2
    tb = kb // 2
    ndr = tp - tb

    s = np.sign(np.clip(w, -1.0, 1.0)).astype(ml_dtypes.float8_e4m3fn)
    # pack sT[i, o] so that row (t*P + p), slot j == logical i = 256t+128j+p
    sT = np.ascontiguousarray(s.T)  # [in, out]
    w_pack = np.ascontiguousarray(
        sT.reshape(tp, 2, P, -1).transpose(0, 2, 1, 3).reshape(tp * P, 2, -1))

    xT = np.ascontiguousarray(x.T)  # [in, N]
    kcut = kb * P
    xb_full = xT[:kcut].astype(ml_dtypes.bfloat16)  # [kcut, N]
    xq_flat = xT[kcut:].astype(ml_dtypes.float8_e4m3fn)  # [in-kcut, N]
    xq_full = np.ascontiguousarray(
        xq_flat.reshape(ndr, 2, P, -1).transpose(0, 2, 1, 3)
        .reshape(ndr * P, 2, -1))

    maps = []
    for c in range(n_cores):
        m = {"w": w_pack}
        if kb > 0:
            m["xb"] = np.ascontiguousarray(xb_full[:, c * ns:(c + 1) * ns])
        if ndr > 0:
            m["xq"] = np.ascontiguousarray(xq_full[:, :, c * ns:(c + 1) * ns])
        maps.append(m)
    return maps


def run(x, weights_real, trace=False, kb=KB, **kwargs):
    nc = build_nc(N_FULL // N_CORES, IN_CH, OUT_CH, kb)
    in_maps = prep_in_maps(x, weights_real, kb)
    res = run_bass_kernel_spmd(nc, in_maps, list(range(N_CORES)), trace=trace,
                               **kwargs)
    out = np.concatenate(
        [np.asarray(res.results[c]["out"]) for c in range(N_CORES)], axis=0
    )
    return np.ascontiguousarray(out.astype(np.float32)), res


def kernel(x, weights_real):
    out, _ = run(x, weights_real)
    return out


# revision 33
# speedup vs baseline: 1.0151x; 1.0151x over previous
"""BinLinear (sign-quantized linear) Trainium2 kernel, mixed bf16/fp8.

Computes out = x @ sign(clip(w, -1, 1)).T for x[8192, 4096], w[4096, 4096],
data-parallel over the 8 NeuronCores (each core takes 1024 rows of x and the
full weight matrix).

The PE moving-operand rate is one column per cycle regardless of dtype, so
the bf16 baseline (~462us) sits at the bf16 roofline. The only faster mode on
TRN2 is fp8 DoubleRow (2 fp8 weights per cell, 256-deep contraction per
matmul, measured at the same 216ns issue gap as a 128-deep bf16 matmul =
2x MACs/cycle). Pure e4m3 x quantization gives 2.64e-2 relative error --
over the 2e-2 budget -- so the contraction is split:

  - KB=16 of the 32 k-tiles (128 each) run in bf16: x bf16 stationary,
    sign(w) fp8 moving (fp8 moving runs at bf16 speed and halves weight DMA;
    sign() is computed on host so no on-device activation is needed).
  - the remaining 16 k-tiles run as 8 fp8 DoubleRow pair-tiles: x e4m3
    stationary [128,2,128], sign(w) e4m3 moving [128,2,512] (+-1 exact).

Exact full-output error (deterministic): rel_norm 1.881e-2, absmax-rel
1.46e-2, vs the 2e-2 gate.

Schedule: 7 warmup matmuls on a zeroed tile (memset on the otherwise-idle
Vector engine) bridge the initial DMA wait so the HAM clock-gate (half rate
until ~3.4us of sustained PE activity) is at 8/8 when the real stream
begins; bf16 x tiles load as two 512-column halves (1KB DMA packets run
~40% faster than the 512B packets smaller chunks produce) alternating
between the Activation and sync DGE queues so neither queue's ~150GB/s
saturates during ob0; weight tiles stream on the sync DGE queue with 4-tile
lookahead; odd output blocks run their pair-tiles in reverse (DR first) so
consecutive blocks share the PE perf mode at the boundary -- each bf16<->DR
switch costs ~200-600ns of PE pipeline drain; PSUM drains alternate
VectorE/ScalarE and their output DMAs use the gpsimd SWDGE queue so they
never block weight fetches; outputs ship as bf16 (host upcasts, ~1e-3 error
contribution); the final output block runs from prefetched SBUF tiles as a
DR pass opening all 8 bank chains (seamless from ob6) then a bf16 pass
closing them with drains spread across the pass, and the last bank is two
complete 256-column chains (two interleaved open accumulation groups in
one bank corrupt results, so each half runs start->stop by itself) whose
final drain splits across both copy engines and both HW DGE queues so only
~2x32KB of parallel copy+DMA trails the final matmul.
"""

import numpy as np
import ml_dtypes

import concourse.mybir as mybir
import concourse.tile as tile
from concourse import bacc
from concourse.bass_utils import run_bass_kernel_spmd

N_CORES = 8
N_FULL, IN_CH, OUT_CH = 8192, 4096, 4096
P = 128
OBLK = 512
KB = 16  # k-tiles (of 32) done in bf16; must be even. rest = fp8 DoubleRow.

DR = mybir.MatmulPerfMode.DoubleRow
COPY_FN = mybir.ActivationFunctionType.Copy


def build_nc(ns, in_ch, out_ch, kb):
    kt = in_ch // P        # total 128-k-tiles (32)
    tp = kt // 2           # pair tiles (16)
    tb = kb // 2           # pair tiles fully in bf16
    ndr = tp - tb          # DoubleRow pair tiles
    msub = ns // P         # PSUM banks in flight (8)
    nob = out_ch // OBLK   # output-channel blocks (8)
    assert kb % 2 == 0 and msub <= 8

    nc = bacc.Bacc("TRN2", target_bir_lowering=False, debug=False)
    if kb > 0:
        xb_d = nc.dram_tensor("xb", [kb * P, ns], mybir.dt.bfloat16,
                              kind="ExternalInput")
    if ndr > 0:
        xq_d = nc.dram_tensor("xq", [ndr * P, 2, ns], mybir.dt.float8e4,
                              kind="ExternalInput")
    w_d = nc.dram_tensor("w", [tp * P, 2, out_ch], mybir.dt.float8e4,
                         kind="ExternalInput")
    out_d = nc.dram_tensor("out", [ns, out_ch], mybir.dt.bfloat16,
                           kind="ExternalOutput")

    with tile.TileContext(nc) as tc:
        with (
            tc.tile_pool(name="xbpool", bufs=1) as xbpool,
            tc.tile_pool(name="xqpool", bufs=1) as xqpool,
            tc.tile_pool(name="wpool", bufs=20) as wpool,
            tc.tile_pool(name="wlpool", bufs=1) as wlpool,
            tc.tile_pool(name="opool", bufs=8) as opool,
            tc.tile_pool(name="const", bufs=1) as const,
            tc.tile_pool(name="pspool", bufs=1, space="PSUM") as pspool,
        ):
            # HAM warmup: self-contained matmuls on a zeroed tile run during
            # the initial input-DMA wait so the PE clock-gate (4/8 = half
            # rate until ~3.4us of sustained activity) reaches 8/8 before
            # the real stream begins. The memset rides the Vector engine
            # (idle until the first drain) so the warmups start as early as
            # possible after the framework preamble; 7 warmups keep the PE
            # busy through the DMA spin-up window so the gate never
            # re-throttles.
            wz = const.tile([P, OBLK], mybir.dt.bfloat16, name="wz")
            nc.vector.memset(wz[:], 0.0)
            warm_ps = pspool.tile([P, OBLK], mybir.dt.float32, name="ps_7")
            for _ in range(7):
                nc.tensor.matmul(warm_ps[:], wz[:, :P], wz[:],
                                 start=True, stop=True)

            x_b = [None] * kb    # bf16 x tiles [P, ns]
            x_q = [None] * ndr   # fp8 pair x tiles [P, 2, ns]
            wlast = [None] * tp  # prefetched last-block w tiles

            def load_x(t):
                # lazily load the x tiles needed by pair-tile t. Loads go on
                # the Activation engine's HW DGE queue so ob0's x streaming
                # never delays weight fetches on the sync queue. bf16 tiles
                # load as two 512-column halves: DMA engine rate is packet-
                # size-bound (1KB packets ~210GB/s aggregate vs 512B ~150),
                # and half-tile granularity keeps arrival ahead of the
                # matmul stream from the very first tile.
                if t >= tp:
                    return
                if t < tb:
                    for j in (0, 1):
                        k = 2 * t + j
                        if x_b[k] is None:
                            xt = xbpool.tile([P, ns], mybir.dt.bfloat16,
                                             name=f"xb_{k}")
                            eng = nc.sync if k % 2 == 1 else nc.scalar
                            cw = ns // 2
                            for c in range(2):
                                eng.dma_start(
                                    out=xt[:, c * cw:(c + 1) * cw],
                                    in_=xb_d[k * P:(k + 1) * P,
                                             c * cw:(c + 1) * cw])
                            x_b[k] = xt
                else:
                    td = t - tb
                    if x_q[td] is None:
                        xt = xqpool.tile([P, 2, ns], mybir.dt.float8e4,
                                         name=f"xq_{td}")
                        nc.scalar.dma_start(
                            out=xt[:], in_=xq_d[td * P:(td + 1) * P, :, :])
                        x_q[td] = xt

            def drain(m, psum, c0):
                # copies alternate VectorE/ScalarE so the 8-bank chain halves
                # (splitting each bank across BOTH engines measures worse --
                # it doubles each engine's copy-queue depth and delays the
                # late banks); output DMAs ride the gpsimd SWDGE queue so a
                # drain waiting on its copy never blocks weight fetches on
                # the sync queue
                ot = opool.tile([P, OBLK], mybir.dt.bfloat16, name="ot")
                if m % 2 == 1:
                    nc.scalar.activation(ot[:], psum[:], COPY_FN)
                else:
                    nc.vector.tensor_copy(ot[:], psum[:])
                nc.gpsimd.dma_start(
                    out=out_d[m * P:(m + 1) * P, c0:c0 + OBLK],
                    in_=ot[:])

            # flat (ob, t) job list with w DMAs emitted 4 tiles ahead so the
            # next block's first weight fetches are queued before the current
            # block's drains. Odd blocks run their pair-tiles in reverse
            # (DR first) so consecutive blocks share the PE perf mode at the
            # boundary: each bf16<->DR switch costs ~200-600ns of PE pipeline
            # drain, and alternating halves the switch count.
            jobs = [(ob, t)
                    for ob in range(nob - 1)
                    for t in (range(tp) if ob % 2 == 0
                              else range(tp - 1, -1, -1))]
            wtiles = {}

            def fetch(idx):
                if idx < len(jobs):
                    ob, t = jobs[idx]
                    wt = wpool.tile([P, 2, OBLK], mybir.dt.float8e4, name="wt")
                    nc.sync.dma_start(
                        out=wt[:],
                        in_=w_d[t * P:(t + 1) * P, :, ob * OBLK:(ob + 1) * OBLK])
                    wtiles[(ob, t)] = wt

            fetch(0)
            load_x(0)
            load_x(1)
            for i in range(1, 4):
                fetch(i)

            psums = None
            for idx, (ob, t) in enumerate(jobs):
                c0 = ob * OBLK
                first = idx % tp == 0       # first pair-tile of the block
                last = idx % tp == tp - 1   # last pair-tile of the block
                if first:
                    psums = [
                        pspool.tile([P, OBLK], mybir.dt.float32, name=f"ps_{m}")
                        for m in range(msub)
                    ]
                fetch(idx + 4)
                if ob == 0:
                    load_x(t + 2)
                wt = wtiles.pop((ob, t))
                if ob == nob - 3:
                    # prefetch last block's w two blocks early (ACT DGE queue) so
                    # its consolidated DMA wait resolves well before the last
                    # block starts m-outer from SBUF
                    wl = wlpool.tile([P, 2, OBLK], mybir.dt.float8e4,
                                     name=f"wl_{t}")
                    nc.scalar.dma_start(
                        out=wl[:],
                        in_=w_d[t * P:(t + 1) * P, :, (nob - 1) * OBLK:])
                    wlast[t] = wl
                if t < tb:
                    for j in (0, 1):
                        k = 2 * t + j
                        for m in range(msub):
                            nc.tensor.matmul(
                                psums[m][:],
                                x_b[k][:, m * P:(m + 1) * P],
                                wt[:, j:j + 1, :],
                                start=(first and j == 0),
                                stop=(last and j == 1),
                            )
                else:
                    td = t - tb
                    for m in range(msub):
                        nc.tensor.matmul(
                            psums[m][:],
                            x_q[td][:, :, m * P:(m + 1) * P],
                            wt[:],
                            start=first,
                            stop=last,
                            perf_mode=DR,
                        )
                if last:
                    for m in range(msub):
                        drain(m, psums[m], c0)

            # Last block, from prefetched SBUF tiles, as TWO passes over the
            # PSUM regions: a DR pass (seamless entry -- block ob=6 ends in
            # DR mode) opening all chains, then a bf16 pass closing them,
            # with each region's drain right after its bf16 chain so drains
            # spread across the pass. One perf-mode switch for the whole
            # block instead of one per m-group. The last bank is split into
            # two 256-column regions so only a 64KB copy+DMA trails the
            # final matmul (256 is the narrowest width whose matmuls stay
            # ahead of their LDWEIGHTS); its output DMAs ride the
            # low-latency sync queue (weights are done by now).
            c0 = (nob - 1) * OBLK
            psums = [
                pspool.tile([P, OBLK], mybir.dt.float32, name=f"ps_{m}")
                for m in range(msub)
            ]
            h = OBLK // 2

            def last_dr(ps_ap, m, lo, hi, start, stop):
                for td in range(ndr):
                    nc.tensor.matmul(
                        ps_ap,
                        x_q[td][:, :, m * P:(m + 1) * P],
                        wlast[tb + td][:, :, lo:hi],
                        start=(start and td == 0),
                        stop=(stop and td == ndr - 1),
                        perf_mode=DR,
                    )

            def last_bf(ps_ap, m, lo, hi, start, stop):
                for t in range(tb):
                    for j in (0, 1):
                        k = 2 * t + j
                        nc.tensor.matmul(
                            ps_ap,
                            x_b[k][:, m * P:(m + 1) * P],
                            wlast[t][:, j:j + 1, lo:hi],
                            start=(start and t == 0 and j == 0),
                            stop=(stop and t == tb - 1 and j == 1),
                        )

            # full banks m0..6: DR pass opens all chains (seamless entry --
            # block ob=6 ends in DR mode), bf16 pass closes them with each
            # bank's drain right after its chain so drains spread out
            for m in range(msub - 1):
                last_dr(psums[m][:, :], m, 0, OBLK, start=True, stop=False)
            for m in range(msub - 1):
                last_bf(psums[m][:, :], m, 0, OBLK, start=False, stop=True)
                drain(m, psums[m], c0)
            # bank 7: two complete 256-column chains (interleaving two open
            # accumulation groups in ONE bank corrupts the first's results,
            # so each half runs start->stop by itself). Chain A runs
            # bf16-first (seamless from the bf16 pass), chain B DR-first
            # (seamless from A), so only a 64KB copy+DMA on the sync queue
            # trails the final matmul and B's tail runs at bf16 cadence.
            mlast = msub - 1
            last_bf(psums[mlast][:, :h], mlast, 0, h, start=True, stop=False)
            last_dr(psums[mlast][:, :h], mlast, 0, h, start=False, stop=True)
            ot_a = opool.tile([P, h], mybir.dt.bfloat16, name="ot_a")
            nc.vector.tensor_copy(ot_a[:], psums[mlast][:, :h])
            nc.sync.dma_start(
                out=out_d[mlast * P:(mlast + 1) * P, c0:c0 + h],
                in_=ot_a[:])
            last_dr(psums[mlast][:, h:], mlast, h, OBLK, start=True, stop=False)
            last_bf(psums[mlast][:, h:], mlast, h, OBLK, start=False, stop=True)
            # the very last drain splits across both copy engines with two
            # pipelined sync DMAs so the post-final-matmul tail is minimal
            q = h // 2
            ot_b = opool.tile([P, h], mybir.dt.bfloat16, name="ot_b")
            nc.vector.tensor_copy(ot_b[:, :q], psums[mlast][:, h:h + q])
            nc.scalar.activation(ot_b[:, q:], psums[mlast][:, h + q:], COPY_FN)
            nc.sync.dma_start(
                out=out_d[mlast * P:(mlast + 1) * P, c0 + h:c0 + h + q],
                in_=ot_b[:, :q])
            nc.scalar.dma_start(
                out=out_d[mlast * P:(mlast + 1) * P, c0 + h + q:c0 + OBLK],
                in_=ot_b[:, q:])
    nc.compile()
    return nc


def prep_in_maps(x, weights_real, kb, n_cores=N_CORES):
    x = np.asarray(x, dtype=np.float32)
    w = np.asarray(weights_real, dtype=np.float32)
    ns = x.shape[0] // n_cores
    in_ch = x.shape[1]
    kt = in_ch // P
    tp = kt // 2
    tb = kb // 2
    ndr = tp - tb

    s = np.sign(np.clip(w, -1.0, 1.0)).astype(ml_dtypes.float8_e4m3fn)
    # pack sT[i, o] so that row (t*P + p), slot j == logical i = 256t+128j+p
    sT = np.ascontiguousarray(s.T)  # [in, out]
    w_pack = np.ascontiguousarray(
        sT.reshape(tp, 2, P, -1).transpose(0, 2, 1, 3).reshape(tp * P, 2, -1))

    xT = np.ascontiguousarray(x.T)  # [in, N]
    kcut = kb * P
    xb_full = xT[:kcut].astype(ml_dtypes.bfloat16)  # [kcut, N]
    xq_flat = xT[kcut:].astype(ml_dtypes.float8_e4m3fn)  # [in-kcut, N]
    xq_full = np.ascontiguousarray(
        xq_flat.reshape(ndr, 2, P, -1).transpose(0, 2, 1, 3)
        .reshape(ndr * P, 2, -1))

    maps = []
    for c in range(n_cores):
        m = {"w": w_pack}
        if kb > 0:
            m["xb"] = np.ascontiguousarray(xb_full[:, c * ns:(c + 1) * ns])
        if ndr > 0:
            m["xq"] = np.ascontiguousarray(xq_full[:, :, c * ns:(c + 1) * ns])
        maps.append(m)
    return maps


def run(x, weights_real, trace=False, kb=KB, **kwargs):
    nc = build_nc(N_FULL // N_CORES, IN_CH, OUT_CH, kb)
    in_maps = prep_in_maps(x, weights_real, kb)
    res = run_bass_kernel_spmd(nc, in_maps, list(range(N_CORES)), trace=trace,
                               **kwargs)
    out = np.concatenate(
        [np.asarray(res.results[c]["out"]) for c in range(N_CORES)], axis=0
    )
    return np.ascontiguousarray(out.astype(np.float32)), res


def kernel(x, weights_real):
    out, _ = run(x, weights_real)
    return out
